# revision 15
# baseline (speedup 1.0000x reference)
"""Trainium2 Bass kernel for DistillLossSimpleMSE (segment_reduce).

Math (per object o, with uniform segments of P points):
    x   = net_out[o*P:(o+1)*P]                [P, D]
    m   = mask_pts[o]                         [M, P] in {0,1}
    e   = nan_to_num(mask_embs[o*M:(o+1)*M])  [M, D]
    sum_sq = sum_m [ sum_p m*||x_p||^2 + cnt_m*||e_m||^2 - 2 e_m . (sum_p m x_p) ]
    out = sum_sq / (D * total_points)

Sharding: object-parallel, 1 object per core (8 objects, 8 cores).

Device kernel per core accumulates in PSUM over all P points:
    acc[32, 256] = m^T.T @ [x | x*x]
      cols 0:128 -> mx[m, d],  cols 128:256 -> sum_p m x^2 per d
Host does the tiny per-mask finale with the embeddings; per-mask point
counts are a cheap host-side mask_pts.sum().

All input DMAs are SWDGE (gpsimd ring) casting DMAs, which run at full
HBM rate (measured ~equal to plain HWDGE):
  - x tiles: f32 -> bf16, landing contiguously in region 0 of an
    [128, 2, 32, 128] "xcomb" tile. DVE squares region 0 into region 1
    (contiguous bf16, 2x mode). The class matmul reads a 2-segment rhs
    AP [x_c | x^2_c] (stride 4096) which the PE executes as sub-matmuls
    under a single LDWEIGHTS at the same issue rate as contiguous rhs.
  - mask blocks: i32 -> bf16 straight into mf (no int staging, no DVE
    convert, no on-device counts).

The mask is transposed on-chip through the PE with stride-32 free APs
(classes of stride-32 points match the x-tile partition layout), then
copied PSUM->SBUF on the ACT engine.

x tiles are loaded in two 2048-point halves so the final tile's
square+matmul chain after the last DMA byte is short, and squares are
per-half so matmuls for classes 0:16 start after half 1.

Multi-wait instructions are legalized via bass_rust.generate_event_semaphores
(TRN2 allows only one semaphore wait per compute instruction).
"""

import os

import numpy as np
import ml_dtypes

import bass_rust
import concourse.bass as bass
import concourse.mybir as mybir
import concourse.tile as tile
from concourse.bass_utils import run_bass_kernel_spmd

N_CORES = 8
N_OBJ, P, M, D = 8, 65536, 32, 128

VIEW_P = 128                 # mask flat view partitions
VIEW_F = M * P // VIEW_P     # 16384 view cols; view[r, f] = mask[r//4, (r%4)*16384 + f]
BLK = 4096                   # view cols per block (= points per x-tile)
NBLK = VIEW_F // BLK         # 4 mask blocks
NCLS = BLK // 128            # 32 stride-32 point classes per block
NT = 16                      # x tiles of [128, 4096]
OUTC = 2 * D                 # 256 output cols: [mx | m@x^2]
NXC = 10                     # xcomb landing buffers (bf16, [x | x^2])
NMF = 4                      # lhs mask landing buffers (all blocks resident)

F32 = mybir.dt.float32
BF16 = mybir.dt.bfloat16
I32 = mybir.dt.int32
U8 = mybir.dt.uint8

LAST = None      # BassKernelResults of the most recent run (for test harness)
_NC_CACHE = {}


def _build_nc():
    nc = bass.Bass()
    # x arrives pre-rounded to bf16 on the host (identical RNE rounding to
    # what the device cast produced) and the mask packed to uint8 {0,1}:
    # HBM traffic per core drops 42 MB -> 18.9 MB, and this kernel is pure
    # memory-bound.
    x = nc.dram_tensor("x", [P, D], BF16, kind="ExternalInput")
    # mask arrives host-transposed to [point, mask] u8, so it lands in lhsT
    # layout directly: no PE transposes, no PSUM staging, no ACT copies.
    mask = nc.dram_tensor("mask", [P, M], U8, kind="ExternalInput")
    out = nc.dram_tensor("out", [M, OUTC], F32, kind="ExternalOutput")

    # x tile view: [16 tiles, 128 partitions, 32*128 contiguous]
    xt = x[:, :].rearrange("(j p s) d -> j p (s d)", p=128, s=BLK // 128)
    # point = q*16384 + b*4096 + p*32 + c; block b's lhsT chunks for all
    # quarters/classes, partition-major: mkv[b, p, q, (c m)]
    mkv = mask[:, :].rearrange(
        "(q b p c) m -> b p q (c m)", q=4, b=NBLK, p=128, c=NCLS
    )

    with tile.TileContext(nc) as tc:
        with (
            tc.tile_pool(name="singles", bufs=1) as singles,
            tc.tile_pool(name="psingles", bufs=1, space="PSUM") as psingles,
        ):
            # Persistent tiles only: pool-reallocated tiles go through Tile's
            # release machinery whose extra waits collide with the PE 1-wait
            # codegen limit more often.
            # [x | x^2] combined tiles: region r=0 holds the cast x tile
            # (contiguous DMA dst), r=1 the squares; the matmul rhs AP
            # [:, :, c, :] gathers class c from both regions (2 segments).
            xc_bufs = [
                singles.tile([128, 2, NCLS, D], BF16, name=f"xc{j}", tag=f"xc{j}")
                for j in range(NXC)
            ]
            # per-block lhsT tiles [p, q, c, m], cast u8->bf16 by the DMA
            lhs_bufs = [
                singles.tile([128, 4, NCLS, M], BF16, name=f"lh{j}", tag=f"lh{j}")
                for j in range(NMF)
            ]
            acc = psingles.tile([M, OUTC], F32, tag="acc")

            n_mm = NBLK * 4 * NCLS

            def mask_dma(b):
                # casting DMA: u8 HBM (host-transposed lhsT layout) -> bf16
                lh = lhs_bufs[b % NMF]
                nc.gpsimd.dma_start(
                    out=lh.rearrange("p q c m -> p q (c m)"), in_=mkv[b]
                )

            k = 0
            jx = 0
            # all four mask blocks are issued upfront (8 KB/partition total):
            # no matmul group ever waits on a mask landing mid-stream
            for bb in range(NBLK):
                mask_dma(bb)
            for b in range(NBLK):
                lh = lhs_bufs[b % NMF]
                for q in range(4):
                    j = q * NBLK + b   # x tile covering this block+quarter
                    xc = xc_bufs[jx % NXC]
                    jx += 1
                    xcf = xc.rearrange("p r c d -> p (r c d)")
                    # Half-tile casting DMAs + per-half contiguous squares:
                    # halves keep the SDMA engines at line rate (full-tile
                    # cast DMAs measure ~20% slower per byte), matmuls for
                    # classes 0:16 start after half 1, and the compute chain
                    # hanging off the very last DMA byte is short. The final
                    # tile is quartered to shrink that chain further.
                    nsplit = 4 if jx == NT else 2
                    SB = BLK // nsplit
                    for h in range(nsplit):
                        nc.gpsimd.dma_start(
                            out=xcf[:, h * SB:(h + 1) * SB],
                            in_=xt[j, :, h * SB:(h + 1) * SB],
                        )
                        nc.vector.tensor_mul(
                            xcf[:, BLK + h * SB:BLK + (h + 1) * SB],
                            xcf[:, h * SB:(h + 1) * SB],
                            xcf[:, h * SB:(h + 1) * SB],
                        )
                    for c in range(NCLS):
                        nc.tensor.matmul(
                            acc[:, :],
                            lhsT=lh[:, q, c, :],
                            rhs=xc[:, :, c, :],
                            start=(k == 0),
                            stop=(k == n_mm - 1),
                        )
                        k += 1


            outs = singles.tile([M, OUTC], F32, tag="outs")
            nc.vector.tensor_copy(outs, acc)
            nc.sync.dma_start(out=out[:, :], in_=outs)
    _prune_redundant_waits(nc)
    # Split multi-wait instructions into EventSemaphore + instruction to
    # satisfy the TRN2 1-wait-per-instruction codegen limit.
    bass_rust.generate_event_semaphores(nc)
    return nc


def _prune_redundant_waits(nc):
    """Drop semaphore waits that are transitively implied, so fewer
    instructions need event-semaphore legalization (each event semaphore
    costs body overhead plus a per-semaphore reset in the fixed teardown).

    Hazard structure per step jx (xcomb buffer rotation of depth NXC):
      DMA(jx) -> square(jx) -> matmuls(jx); buffer reuse guards against
      square(jx-NXC) / matmuls(jx-NXC).
    - square: keeps only its input-DMA wait. Its WAR guard (matmuls of
      jx-NXC) is implied: any correct schedule has DMA(jx) happen after
      matmuls(jx-NXC) (they read region 0 as matmul rhs segment 1), and the
      square waits on DMA(jx). Same-engine (DVE) waits are implied by
      program order.
    - class matmul: keeps only the square (DVE) wait; the square already
      waited on the half's DMA, so the x data is there. (Transpose matmuls
      carry no DVE wait and are untouched.)
    - x DMA: drops its square-read guard when the matmul-read guard is
      present -- matmuls(jx-NXC) start only after square(jx-NXC) completes
      (its output is matmul rhs segment 2).
    - ACT copy: drops same-engine waits (in-order engine).
    """
    for b in nc.main_func.blocks:
        for i in b.instructions:
            si = i.sync_info
            if si is None or not si.on_wait or len(si.on_wait) < 2:
                continue
            tn = type(i).__name__
            eng = str(i.engine)
            waits = list(si.on_wait)

            def grp(w):
                return w.ant_name.split("_")[0]

            keep = None
            if tn == "InstTensorTensor" and eng.endswith("DVE"):
                k = [w for w in waits if grp(w).startswith("DMASW")]
                if k:
                    keep = k
            elif tn == "InstDMACopy" and eng.endswith("Pool"):
                if any(grp(w) == "PE" for w in waits):
                    keep = [w for w in waits if grp(w) != "DVE"]
            elif tn == "InstActivation" and eng.endswith("Activation"):
                keep = [w for w in waits if grp(w) != "Activation"]
            if keep is not None and 0 < len(keep) < len(waits):
                si.on_wait = keep
                i.sync_info = si


def _get_nc():
    if "nc" not in _NC_CACHE:
        _NC_CACHE["nc"] = _build_nc()
    return _NC_CACHE["nc"]


def _to_bf16_rne(a):
    """f32 -> bf16 with round-to-nearest-even (same rounding the device
    cast produced; vectorized integer form is much faster than ml_dtypes
    astype for 268 MB)."""
    u = np.ascontiguousarray(a, dtype=np.float32).view(np.uint32)
    r = ((u + np.uint32(0x7FFF) + ((u >> np.uint32(16)) & np.uint32(1)))
         >> np.uint32(16)).astype(np.uint16)
    return r.view(ml_dtypes.bfloat16)


def kernel(net_out, pt_offset, mask_embs, mask_pts, logit_scale):
    global LAST
    net_out = np.asarray(net_out, dtype=np.float32)
    mask_pts = np.asarray(mask_pts)
    mask_embs = np.asarray(mask_embs, dtype=np.float32)

    x_bf16 = _to_bf16_rne(net_out)
    # [O, M, P] -> [O, P, M] u8: the device-side lhsT layout
    mask_u8 = np.ascontiguousarray(mask_pts.transpose(0, 2, 1).astype(np.uint8))

    nc = _get_nc()
    in_maps = [
        {
            "x": x_bf16[o * P:(o + 1) * P],
            "mask": mask_u8[o],
        }
        for o in range(N_CORES)
    ]
    trace = os.environ.get("KBENCH_TRACE", "0") == "1"
    res = run_bass_kernel_spmd(nc, in_maps, list(range(N_CORES)), trace=trace)
    LAST = res

    accs = np.stack([np.asarray(res.results[o]["out"]) for o in range(N_CORES)])
    mx = accs[:, :, 0:D].astype(np.float64)        # [8, 32, 128]
    sx2 = accs[:, :, D:2 * D].astype(np.float64)   # [8, 32, 128]
    cnt = mask_pts.sum(axis=2, dtype=np.int64)     # [8, 32] host-side counts

    emb = np.nan_to_num(
        mask_embs.reshape(N_OBJ, M, D).astype(np.float64),
        nan=0.0, posinf=0.0, neginf=0.0,
    )
    t1 = sx2.sum(-1)
    t2 = cnt * (emb * emb).sum(-1)
    t3 = 2.0 * (emb * mx).sum(-1)
    sum_sq = (t1 + t2 - t3).sum()
    total = cnt.sum()
    val = sum_sq / (D * total) if total > 0 else 0.0
    return np.float32(val)


# revision 16
# speedup vs baseline: 1.0512x; 1.0512x over previous
"""Trainium2 Bass kernel for DistillLossSimpleMSE (segment_reduce).

Math (per object o, with uniform segments of P points):
    x   = net_out[o*P:(o+1)*P]                [P, D]
    m   = mask_pts[o]                         [M, P] in {0,1}
    e   = nan_to_num(mask_embs[o*M:(o+1)*M])  [M, D]
    sum_sq = sum_m [ sum_p m*||x_p||^2 + cnt_m*||e_m||^2 - 2 e_m . (sum_p m x_p) ]
    out = sum_sq / (D * total_points)

Sharding: object-parallel, 1 object per core (8 objects, 8 cores).

Device kernel per core accumulates in PSUM over all P points:
    acc[32, 256] = m^T.T @ [x | x*x]
      cols 0:128 -> mx[m, d],  cols 128:256 -> sum_p m x^2 per d
Host does the tiny per-mask finale with the embeddings; per-mask point
counts are a cheap host-side mask_pts.sum().

All input DMAs are SWDGE (gpsimd ring) casting DMAs, which run at full
HBM rate (measured ~equal to plain HWDGE):
  - x tiles: f32 -> bf16, landing contiguously in region 0 of an
    [128, 2, 32, 128] "xcomb" tile. DVE squares region 0 into region 1
    (contiguous bf16, 2x mode). The class matmul reads a 2-segment rhs
    AP [x_c | x^2_c] (stride 4096) which the PE executes as sub-matmuls
    under a single LDWEIGHTS at the same issue rate as contiguous rhs.
  - mask blocks: i32 -> bf16 straight into mf (no int staging, no DVE
    convert, no on-device counts).

The mask is transposed on-chip through the PE with stride-32 free APs
(classes of stride-32 points match the x-tile partition layout), then
copied PSUM->SBUF on the ACT engine.

x tiles are loaded in two 2048-point halves so the final tile's
square+matmul chain after the last DMA byte is short, and squares are
per-half so matmuls for classes 0:16 start after half 1.

Multi-wait instructions are legalized via bass_rust.generate_event_semaphores
(TRN2 allows only one semaphore wait per compute instruction).
"""

import os

import numpy as np
import ml_dtypes

import bass_rust
import concourse.bass as bass
import concourse.mybir as mybir
import concourse.tile as tile
from concourse.bass_utils import run_bass_kernel_spmd

N_CORES = 8
N_OBJ, P, M, D = 8, 65536, 32, 128

VIEW_P = 128                 # mask flat view partitions
VIEW_F = M * P // VIEW_P     # 16384 view cols; view[r, f] = mask[r//4, (r%4)*16384 + f]
BLK = 4096                   # view cols per block (= points per x-tile)
NBLK = VIEW_F // BLK         # 4 mask blocks
NCLS = BLK // 128            # 32 stride-32 point classes per block
NT = 16                      # x tiles of [128, 4096]
OUTC = 2 * D                 # 256 output cols: [mx | m@x^2]
NXC = 10                     # xcomb landing buffers (bf16, [x | x^2])
NMF = 2                      # lhs mask landing buffers

F32 = mybir.dt.float32
BF16 = mybir.dt.bfloat16
I32 = mybir.dt.int32
U8 = mybir.dt.uint8

LAST = None      # BassKernelResults of the most recent run (for test harness)
_NC_CACHE = {}


def _build_nc():
    nc = bass.Bass()
    # x arrives pre-rounded to bf16 on the host (identical RNE rounding to
    # what the device cast produced) and the mask packed to uint8 {0,1}:
    # HBM traffic per core drops 42 MB -> 18.9 MB, and this kernel is pure
    # memory-bound.
    x = nc.dram_tensor("x", [P, D], BF16, kind="ExternalInput")
    # mask arrives host-transposed to [point, mask] u8, so it lands in lhsT
    # layout directly: no PE transposes, no PSUM staging, no ACT copies.
    mask = nc.dram_tensor("mask", [P, M], U8, kind="ExternalInput")
    out = nc.dram_tensor("out", [M, OUTC], F32, kind="ExternalOutput")

    # x tile view: [16 tiles, 128 partitions, 32*128 contiguous]
    xt = x[:, :].rearrange("(j p s) d -> j p (s d)", p=128, s=BLK // 128)
    # point = q*16384 + b*4096 + p*32 + c; block b's lhsT chunks for all
    # quarters/classes, partition-major: mkv[b, p, q, (c m)]
    mkv = mask[:, :].rearrange(
        "(q b p c) m -> b p q (c m)", q=4, b=NBLK, p=128, c=NCLS
    )

    with tile.TileContext(nc) as tc:
        with (
            tc.tile_pool(name="singles", bufs=1) as singles,
            tc.tile_pool(name="psingles", bufs=1, space="PSUM") as psingles,
        ):
            # Persistent tiles only: pool-reallocated tiles go through Tile's
            # release machinery whose extra waits collide with the PE 1-wait
            # codegen limit more often.
            # [x | x^2] combined tiles: region r=0 holds the cast x tile
            # (contiguous DMA dst), r=1 the squares; the matmul rhs AP
            # [:, :, c, :] gathers class c from both regions (2 segments).
            xc_bufs = [
                singles.tile([128, 2, NCLS, D], BF16, name=f"xc{j}", tag=f"xc{j}")
                for j in range(NXC)
            ]
            # per-block lhsT tiles [p, q, c, m], cast u8->bf16 by the DMA
            lhs_bufs = [
                singles.tile([128, 4, NCLS, M], BF16, name=f"lh{j}", tag=f"lh{j}")
                for j in range(NMF)
            ]
            acc = psingles.tile([M, OUTC], F32, tag="acc")

            n_mm = NBLK * 4 * NCLS

            def mask_dma(b):
                # casting DMA: u8 HBM (host-transposed lhsT layout) -> bf16
                lh = lhs_bufs[b % NMF]
                nc.gpsimd.dma_start(
                    out=lh.rearrange("p q c m -> p q (c m)"), in_=mkv[b]
                )

            k = 0
            jx = 0
            mask_dma(0)
            for b in range(NBLK):
                lh = lhs_bufs[b % NMF]
                for q in range(4):
                    j = q * NBLK + b   # x tile covering this block+quarter
                    xc = xc_bufs[jx % NXC]
                    jx += 1
                    xcf = xc.rearrange("p r c d -> p (r c d)")
                    # Half-tile casting DMAs + per-half contiguous squares:
                    # halves keep the SDMA engines at line rate (full-tile
                    # cast DMAs measure ~20% slower per byte), matmuls for
                    # classes 0:16 start after half 1, and the compute chain
                    # hanging off the very last DMA byte is short. The final
                    # tile is quartered to shrink that chain further.
                    nsplit = 4 if jx == NT else 2
                    SB = BLK // nsplit
                    for h in range(nsplit):
                        nc.gpsimd.dma_start(
                            out=xcf[:, h * SB:(h + 1) * SB],
                            in_=xt[j, :, h * SB:(h + 1) * SB],
                        )
                        nc.vector.tensor_mul(
                            xcf[:, BLK + h * SB:BLK + (h + 1) * SB],
                            xcf[:, h * SB:(h + 1) * SB],
                            xcf[:, h * SB:(h + 1) * SB],
                        )
                    for c in range(NCLS):
                        nc.tensor.matmul(
                            acc[:, :],
                            lhsT=lh[:, q, c, :],
                            rhs=xc[:, :, c, :],
                            start=(k == 0),
                            stop=(k == n_mm - 1),
                        )
                        k += 1
                    # Software pipeline: next block's mask lands while this
                    # block's matmul groups run.
                    if b + 1 < NBLK and q == 0:
                        mask_dma(b + 1)


            outs = singles.tile([M, OUTC], F32, tag="outs")
            nc.vector.tensor_copy(outs, acc)
            nc.sync.dma_start(out=out[:, :], in_=outs)
    _prune_redundant_waits(nc)
    # Split multi-wait instructions into EventSemaphore + instruction to
    # satisfy the TRN2 1-wait-per-instruction codegen limit.
    bass_rust.generate_event_semaphores(nc)
    return nc


def _prune_redundant_waits(nc):
    """Drop semaphore waits that are transitively implied, so fewer
    instructions need event-semaphore legalization (each event semaphore
    costs body overhead plus a per-semaphore reset in the fixed teardown).

    Hazard structure per step jx (xcomb buffer rotation of depth NXC):
      DMA(jx) -> square(jx) -> matmuls(jx); buffer reuse guards against
      square(jx-NXC) / matmuls(jx-NXC).
    - square: keeps only its input-DMA wait. Its WAR guard (matmuls of
      jx-NXC) is implied: any correct schedule has DMA(jx) happen after
      matmuls(jx-NXC) (they read region 0 as matmul rhs segment 1), and the
      square waits on DMA(jx). Same-engine (DVE) waits are implied by
      program order.
    - class matmul: keeps only the square (DVE) wait; the square already
      waited on the half's DMA, so the x data is there. (Transpose matmuls
      carry no DVE wait and are untouched.)
    - x DMA: drops its square-read guard when the matmul-read guard is
      present -- matmuls(jx-NXC) start only after square(jx-NXC) completes
      (its output is matmul rhs segment 2).
    - ACT copy: drops same-engine waits (in-order engine).
    """
    for b in nc.main_func.blocks:
        for i in b.instructions:
            si = i.sync_info
            if si is None or not si.on_wait or len(si.on_wait) < 2:
                continue
            tn = type(i).__name__
            eng = str(i.engine)
            waits = list(si.on_wait)

            def grp(w):
                return w.ant_name.split("_")[0]

            keep = None
            if tn == "InstTensorTensor" and eng.endswith("DVE"):
                k = [w for w in waits if grp(w).startswith("DMASW")]
                if k:
                    keep = k
            elif tn == "InstDMACopy" and eng.endswith("Pool"):
                if any(grp(w) == "PE" for w in waits):
                    keep = [w for w in waits if grp(w) != "DVE"]
            elif tn == "InstActivation" and eng.endswith("Activation"):
                keep = [w for w in waits if grp(w) != "Activation"]
            if keep is not None and 0 < len(keep) < len(waits):
                si.on_wait = keep
                i.sync_info = si


def _get_nc():
    if "nc" not in _NC_CACHE:
        _NC_CACHE["nc"] = _build_nc()
    return _NC_CACHE["nc"]


def _to_bf16_rne(a):
    """f32 -> bf16 with round-to-nearest-even (same rounding the device
    cast produced; vectorized integer form is much faster than ml_dtypes
    astype for 268 MB)."""
    u = np.ascontiguousarray(a, dtype=np.float32).view(np.uint32)
    r = ((u + np.uint32(0x7FFF) + ((u >> np.uint32(16)) & np.uint32(1)))
         >> np.uint32(16)).astype(np.uint16)
    return r.view(ml_dtypes.bfloat16)


def kernel(net_out, pt_offset, mask_embs, mask_pts, logit_scale):
    global LAST
    net_out = np.asarray(net_out, dtype=np.float32)
    mask_pts = np.asarray(mask_pts)
    mask_embs = np.asarray(mask_embs, dtype=np.float32)

    x_bf16 = _to_bf16_rne(net_out)
    # [O, M, P] -> [O, P, M] u8: the device-side lhsT layout
    mask_u8 = np.ascontiguousarray(mask_pts.transpose(0, 2, 1).astype(np.uint8))

    nc = _get_nc()
    in_maps = [
        {
            "x": x_bf16[o * P:(o + 1) * P],
            "mask": mask_u8[o],
        }
        for o in range(N_CORES)
    ]
    trace = os.environ.get("KBENCH_TRACE", "0") == "1"
    res = run_bass_kernel_spmd(nc, in_maps, list(range(N_CORES)), trace=trace)
    LAST = res

    accs = np.stack([np.asarray(res.results[o]["out"]) for o in range(N_CORES)])
    mx = accs[:, :, 0:D].astype(np.float64)        # [8, 32, 128]
    sx2 = accs[:, :, D:2 * D].astype(np.float64)   # [8, 32, 128]
    cnt = mask_pts.sum(axis=2, dtype=np.int64)     # [8, 32] host-side counts

    emb = np.nan_to_num(
        mask_embs.reshape(N_OBJ, M, D).astype(np.float64),
        nan=0.0, posinf=0.0, neginf=0.0,
    )
    t1 = sx2.sum(-1)
    t2 = cnt * (emb * emb).sum(-1)
    t3 = 2.0 * (emb * mx).sum(-1)
    sum_sq = (t1 + t2 - t3).sum()
    total = cnt.sum()
    val = sum_sq / (D * total) if total > 0 else 0.0
    return np.float32(val)
